# revision 26
# baseline (speedup 1.0000x reference)
"""AdafuseNet multi-view heatmap fusion kernel for 8 TRN2 NeuronCores.

Pure data parallel: 32 batches sharded 4-per-core (8 bv-slices of
(17,128,128) heatmaps per core). v2: bf16 datapath (rel err ~2e-3,
gate is 2e-2), host-side [S,H,J,W] transpose so all big DMAs are
contiguous, two-batch-group pipeline to hide the per-group stats /
camera-math serial chain, bf16 full-rate PE matmuls.

Per core, per group g (batches {2g, 2g+1} = slices 4g..4g+3):
  stage 1: exp(hm/T) -> per-joint column sums via PE matmuls,
           per-joint max via DVE reduce + PE transpose
  tiny math: 3x3 camera inverses, fundamental matrices, epipolar
           distances, view-weight sigmoid - strided DVE ops on [2,2,*]
  fusion:  fused = c0*hm0 + c1*hm1 (per-joint scalars), bf16 out
  stage 2: soft-argmax on fused -> output coords
"""
import os
import sys

for _p in (
    "/root/.axon_site",
    "/root/.axon_site/_ro/trn_rl_repo",
    "/root/.axon_site/_ro/pypackages",
    "/opt/trn_rl_repo",
    "/opt/pypackages",
):
    if os.path.isdir(_p) and _p not in sys.path:
        sys.path.append(_p)

import numpy as np
import ml_dtypes
import concourse.bass as bass
import concourse.tile as tile
from concourse import bacc
from concourse import mybir
from concourse.alu_op_type import AluOpType
from contextlib import ExitStack

B, V, J, H, W = 32, 2, 17, 128, 128
NC_ = 8
BPC = B // NC_        # 4 batches per core
S = BPC * V           # 8 bv-slices per core
NG = 2                # batch groups per core
BPG = BPC // NG       # 2 batches per group
SPG = BPG * V         # 4 slices per group
TINV = 20.0           # 1 / softmax_temp
EPS = 1e-12
F32 = mybir.dt.float32
BF16 = mybir.dt.bfloat16
NPBF = ml_dtypes.bfloat16
FD = J * W            # 2176 free elems per slice
X = mybir.AxisListType.X

# cst column layout (fp32 tensor; cols 0:CBF also cast to a bf16 twin)
C_LHS1 = 0            # 2 cols: [ones | arange]
C_IDEN = 2            # 128 cols: eye(128)
C_L8 = 130            # 4 slots x 8 cols: M1 masked lhsT (pair at 2k)
C_L4 = 162            # 2 slots x 4 cols: stage2 masked lhsT
CBF = 170             # end of bf16-twin region
C_K = 170             # rows 0-1: K per group: g0 18, g1 18
C_T = 206             # rows 0-1: T per group: g0 32, g1 32
C_END = 270


def _ap(base, off, dims):
    """Custom free-dim AP on a tile: keep partition entry, replace free dims.
    dims = [[step, count], ...] in elements relative to base's offset."""
    b = base[:] if not isinstance(base, bass.AP) else base
    return bass.AP(tensor=b.tensor, offset=b.offset + off, ap=[list(b.ap[0])] + dims)


def build_nc():
    nc = bacc.Bacc()
    hms = nc.declare_dram_parameter("hms", [S, H, J, W], BF16, isOutput=False)
    cst = nc.declare_dram_parameter("cst", [128, C_END], F32, isOutput=False)
    out_hm = nc.declare_dram_parameter("out_hm", [S, H, J, W], BF16, isOutput=True)
    out_img = nc.declare_dram_parameter("out_img", [BPC, V, 2, J], F32, isOutput=True)

    MU, ADD, SUB, MX, GT = (AluOpType.mult, AluOpType.add, AluOpType.subtract,
                            AluOpType.max, AluOpType.is_gt)
    ACT = mybir.ActivationFunctionType

    with tile.TileContext(nc) as tc, ExitStack() as ctx:
        consts = ctx.enter_context(tc.tile_pool(name="consts", bufs=1))
        big = ctx.enter_context(tc.tile_pool(name="big", bufs=2))
        epool = ctx.enter_context(tc.tile_pool(name="epool", bufs=1))
        fpool = ctx.enter_context(tc.tile_pool(name="fpool", bufs=3))
        e2pool = ctx.enter_context(tc.tile_pool(name="e2pool", bufs=2))
        sm = ctx.enter_context(tc.tile_pool(name="sm", bufs=2))
        ps = ctx.enter_context(tc.tile_pool(name="ps", bufs=2, space="PSUM"))

        cst_sb = consts.tile([128, C_END], F32)
        nc.sync.dma_start(out=cst_sb, in_=cst[:])
        cst_bf = consts.tile([128, CBF], BF16)
        nc.scalar.copy(out=cst_bf, in_=cst_sb[:, 0:CBF])
        lhs1 = cst_sb[:, C_LHS1:C_LHS1 + 2]
        iden = cst_sb[:, C_IDEN:C_IDEN + 128]
        iden_bf = cst_bf[:, C_IDEN:C_IDEN + 128]
        lhs8_bf = lambda k: cst_bf[:, C_L8 + 8 * k:C_L8 + 8 * (k + 1)]
        lhs4_bf = lambda k: cst_bf[:, C_L4 + 4 * k:C_L4 + 4 * (k + 1)]
        zb = consts.tile([128, 1], F32)
        nc.vector.memset(zb, 0.0)
        ones_row = consts.tile([1, 128], F32)
        nc.vector.memset(ones_row, 1.0)

        hmpool = ctx.enter_context(tc.tile_pool(name="hmpool", bufs=1))
        hm_gs, mxc_gs = [], []
        for g in range(NG):
            hm_g = hmpool.tile([128, SPG, J, W], BF16, tag=f"hm{g}")
            maxcol_g = hmpool.tile([128, SPG, 18], BF16, tag=f"mxc{g}")
            nc.vector.memset(maxcol_g[:, :, J:18], 0.0)
            for k in range(SPG):
                s = SPG * g + k
                nc.sync.dma_start(out=hm_g[:, k].rearrange("p j w -> p (j w)"),
                                  in_=hms[s].rearrange("h j w -> h (j w)"))
            hm_gs.append(hm_g)
            mxc_gs.append(maxcol_g)
        for g in range(NG):
            # ---- stage 1: exp, maxcol, M1 (loads hoisted above) ----
            hm_g, maxcol_g = hm_gs[g], mxc_gs[g]
            e_g = []
            for k in range(SPG):
                e_s = epool.tile([128, J, W], BF16, tag=f"e{g}_{k}")
                nc.scalar.activation(out=e_s, in_=hm_g[:, k], func=ACT.Exp,
                                     bias=zb, scale=TINV)
                nc.vector.tensor_reduce(out=maxcol_g[:, k, 0:J], in_=hm_g[:, k],
                                        axis=X, op=MX)
                e_g.append(e_s.rearrange("p j w -> p (j w)"))
            m1g = big.tile([2 * SPG, J, W], F32, tag="m1g")
            m1gf = m1g.rearrange("p j w -> p (j w)")
            for c0 in range(0, FD, 512):
                c1 = min(c0 + 512, FD)
                p_m1 = ps.tile([2 * SPG, 512], F32, tag="m1")
                for k in range(SPG):
                    nc.tensor.matmul(p_m1[:, 0:c1 - c0], lhs8_bf(k),
                                     e_g[k][:, c0:c1],
                                     start=(k == 0), stop=(k == SPG - 1))
                nc.scalar.copy(out=m1gf[:, c0:c1], in_=p_m1[:, 0:c1 - c0])

            # maxv over partitions: PE-transpose [128, 72] -> reduce -> [1, 72]
            mc_g = maxcol_g.rearrange("p s j -> p (s j)")
            mt = ps.tile([SPG * 18, 128], BF16, tag="aux")
            nc.tensor.transpose(mt, mc_g, iden_bf)
            mred = sm.tile([SPG * 18, 1], F32, tag="mred")
            nc.vector.tensor_reduce(out=mred, in_=mt, axis=X, op=MX)
            tree1 = sm.tile([1, SPG * 18], F32, tag="tree1")
            nc.sync.dma_start(out=tree1, in_=mred)

            # per-joint transposes + M3
            p_t = ps.tile([128, J, 2 * SPG], F32, tag="ptj")
            for j in range(J):
                nc.tensor.transpose(p_t[:, j], m1g[:, j, :],
                                    iden[0:2 * SPG, 0:2 * SPG])
            csT = big.tile([128, J, 2 * SPG], F32, tag="csT")
            nc.scalar.copy(out=csT, in_=p_t)
            p_p2 = ps.tile([2, J, SPG, 2], F32, tag="p2")
            nc.tensor.matmul(p_p2.rearrange("p j s r -> p (j s r)"), lhs1,
                             csT.rearrange("p j s -> p (j s)"),
                             start=True, stop=True)
            sums2 = sm.tile([2, SPG, 2, J], F32, tag="sums2")
            nc.scalar.copy(out=_ap(sums2, 0, [[1, J], [2 * J, SPG], [J, 2]]),
                           in_=p_p2)

            # gathers to [2, V, J] (partition = batch-in-group)
            S_t = sm.tile([BPG, V, J], F32, tag="S_t")
            ynum = sm.tile([BPG, V, J], F32, tag="ynum")
            xnum = sm.tile([BPG, V, J], F32, tag="xnum")
            conf = sm.tile([BPG, V, J], F32, tag="conf")
            nc.sync.dma_start(out=S_t, in_=_ap(sums2[0:1], 0, [[2 * J, SPG], [1, J]]))
            nc.sync.dma_start(out=ynum, in_=_ap(sums2[0:1], J, [[2 * J, SPG], [1, J]]))
            nc.sync.dma_start(out=xnum, in_=_ap(sums2[1:2], 0, [[2 * J, SPG], [1, J]]))
            nc.sync.dma_start(out=conf, in_=_ap(tree1[0:1], 0, [[18, SPG], [1, J]]))

            rS = sm.tile([BPG, V, J], F32, tag="rS")
            nc.vector.reciprocal(rS, S_t)
            img = sm.tile([BPG, V, 3, J], F32, tag="img")
            nc.vector.scalar_tensor_tensor(img[:, :, 0], xnum, 4.0, rS,
                                           op0=MU, op1=MU)
            nc.vector.scalar_tensor_tensor(img[:, :, 1], ynum, 4.0, rS,
                                           op0=MU, op1=MU)
            nc.vector.memset(img[:, :, 2], 1.0)

            # mv = where(conf > 0.01, conf, 1e6); inv_mv = 1/mv
            mask = sm.tile([BPG, V, J], F32, tag="mask")
            nc.vector.tensor_scalar(mask, conf, 0.01, None, op0=GT)
            mv = sm.tile([BPG, V, J], F32, tag="mv")
            nc.vector.tensor_tensor(mv, conf, mask, op=MU)
            mnot = sm.tile([BPG, V, J], F32, tag="mnot")
            nc.vector.tensor_scalar(mnot, mask, -1e6, 1e6, op0=MU, op1=ADD)
            nc.vector.tensor_tensor(mv, mv, mnot, op=ADD)
            inv_mv = sm.tile([BPG, V, J], F32, tag="inv_mv")
            nc.vector.reciprocal(inv_mv, mv)

            # ---- camera math on [2, 2, *] tiles ----
            K_cat = cst_sb[0:BPG, C_K + 18 * g:C_K + 18 * (g + 1)].rearrange(
                "b (v e) -> b v e", v=V)
            T_cat = cst_sb[0:BPG, C_T + 32 * g:C_T + 32 * (g + 1)].rearrange(
                "b (v e) -> b v e", v=V)

            K4 = sm.tile([BPG, V, 36], F32, tag="K4")
            src_K = _ap(K_cat, 0, [[9, V], [3, 3], [1, 3]])
            for qa, qb in ((0, 0), (0, 3), (3, 0), (3, 3)):
                nc.vector.tensor_copy(
                    _ap(K4, qa * 6 + qb, [[36, V], [6, 3], [1, 3]]), src_K)
            u1 = sm.tile([BPG, V, 9], F32, tag="u1")
            u2 = sm.tile([BPG, V, 9], F32, tag="u2")
            cof = sm.tile([BPG, V, 9], F32, tag="cof")
            st = [[36, V], [1, 3], [6, 3]]
            nc.vector.tensor_tensor(u1, _ap(K4, 7, st), _ap(K4, 14, st), op=MU)
            nc.vector.tensor_tensor(u2, _ap(K4, 8, st), _ap(K4, 13, st), op=MU)
            nc.vector.tensor_tensor(cof, u1, u2, op=SUB)
            det3 = sm.tile([BPG, V, 3], F32, tag="det3")
            nc.vector.tensor_tensor(det3, _ap(K_cat, 0, [[9, V], [1, 3]]),
                                    _ap(cof, 0, [[9, V], [3, 3]]), op=MU)
            det = sm.tile([BPG, V, 1], F32, tag="det")
            nc.vector.tensor_reduce(out=det, in_=det3, axis=X, op=ADD)
            rdet = sm.tile([BPG, V, 1], F32, tag="rdet")
            nc.vector.reciprocal(rdet, det)
            invK = sm.tile([BPG, V, 9], F32, tag="invK")
            nc.vector.scalar_tensor_tensor(invK, cof, 1.0,
                                           _ap(rdet, 0, [[1, V], [0, 9]]),
                                           op0=MU, op1=MU)
            invK_sw = sm.tile([BPG, V, 9], F32, tag="invK_sw")
            nc.vector.tensor_copy(invK_sw[:, 0], invK[:, 1])
            nc.vector.tensor_copy(invK_sw[:, 1], invK[:, 0])

            r01 = sm.tile([BPG, 9], F32, tag="r01")
            tmp9 = sm.tile([BPG, 9], F32, tag="tmp9")
            for k in range(3):
                dst = r01 if k == 0 else tmp9
                nc.vector.tensor_tensor(dst, _ap(T_cat, k, [[4, 3], [0, 3]]),
                                        _ap(T_cat, 16 + k, [[0, 3], [4, 3]]), op=MU)
                if k:
                    nc.vector.tensor_tensor(r01, r01, tmp9, op=ADD)

            t_t = sm.tile([BPG, V, 3], F32, tag="t_t")
            tmp33 = sm.tile([BPG, 3, 3], F32, tag="tmp33")
            tmp3 = sm.tile([BPG, 3, 1], F32, tag="tmp3")
            nc.vector.tensor_tensor(tmp33, _ap(r01, 0, [[3, 3], [1, 3]]),
                                    _ap(T_cat, 16 + 3, [[0, 3], [4, 3]]), op=MU)
            nc.vector.tensor_reduce(out=tmp3, in_=tmp33, axis=X, op=ADD)
            nc.vector.tensor_tensor(t_t[:, 0], _ap(T_cat, 3, [[4, 3]]),
                                    tmp3[:, :, 0], op=SUB)
            nc.vector.tensor_tensor(tmp33, _ap(r01, 0, [[1, 3], [3, 3]]),
                                    _ap(T_cat, 3, [[0, 3], [4, 3]]), op=MU)
            nc.vector.tensor_reduce(out=tmp3, in_=tmp33, axis=X, op=ADD)
            nc.vector.tensor_tensor(t_t[:, 1], _ap(T_cat, 16 + 3, [[4, 3]]),
                                    tmp3[:, :, 0], op=SUB)

            t_t2 = sm.tile([BPG, V, 6], F32, tag="t_t2")
            nc.vector.tensor_copy(_ap(t_t2, 0, [[6, V], [1, 3]]), t_t)
            nc.vector.tensor_copy(_ap(t_t2, 3, [[6, V], [1, 3]]), t_t)
            r_t2 = sm.tile([BPG, V, 18], F32, tag="r_t2")
            nc.vector.tensor_copy(_ap(r_t2, 0, [[1, 9]]), r01)
            nc.vector.tensor_copy(_ap(r_t2, 9, [[1, 9]]), r01)
            rT = _ap(r01, 0, [[1, 3], [3, 3]])
            nc.vector.tensor_copy(_ap(r_t2, 18, [[3, 3], [1, 3]]), rT)
            nc.vector.tensor_copy(_ap(r_t2, 27, [[3, 3], [1, 3]]), rT)
            Mu1 = sm.tile([BPG, V, 9], F32, tag="Mu1")
            Mu2 = sm.tile([BPG, V, 9], F32, tag="Mu2")
            Mcat = sm.tile([BPG, V, 9], F32, tag="Mcat")
            nc.vector.tensor_tensor(Mu1, _ap(t_t2, 1, [[6, V], [1, 3], [0, 3]]),
                                    _ap(r_t2, 6, [[18, V], [3, 3], [1, 3]]), op=MU)
            nc.vector.tensor_tensor(Mu2, _ap(t_t2, 2, [[6, V], [1, 3], [0, 3]]),
                                    _ap(r_t2, 3, [[18, V], [3, 3], [1, 3]]), op=MU)
            nc.vector.tensor_tensor(Mcat, Mu1, Mu2, op=SUB)

            G = sm.tile([BPG, V, 9], F32, tag="G")
            tmpG = sm.tile([BPG, V, 9], F32, tag="tmpG")
            for p_ in range(3):
                dst = G if p_ == 0 else tmpG
                nc.vector.tensor_tensor(
                    dst, _ap(invK, 3 * p_, [[9, V], [1, 3], [0, 3]]),
                    _ap(Mcat, 3 * p_, [[9, V], [0, 3], [1, 3]]), op=MU)
                if p_:
                    nc.vector.tensor_tensor(G, G, tmpG, op=ADD)
            Fm = sm.tile([BPG, V, 9], F32, tag="Fm")
            tmpF = sm.tile([BPG, V, 9], F32, tag="tmpF")
            for q in range(3):
                dst = Fm if q == 0 else tmpF
                nc.vector.tensor_tensor(
                    dst, _ap(G, q, [[9, V], [3, 3], [0, 3]]),
                    _ap(invK_sw, 3 * q, [[9, V], [0, 3], [1, 3]]), op=MU)
                if q:
                    nc.vector.tensor_tensor(Fm, Fm, tmpF, op=ADD)

            img_sw = sm.tile([BPG, V, 3, J], F32, tag="img_sw")
            nc.vector.tensor_copy(img_sw[:, 0], img[:, 1])
            nc.vector.tensor_copy(img_sw[:, 1], img[:, 0])
            l_t = sm.tile([BPG, V, 3, J], F32, tag="l_t")
            l_tmp = sm.tile([BPG, V, 3, J], F32, tag="l_tmp")
            for n in range(3):
                dst = l_t if n == 0 else l_tmp
                nc.vector.tensor_tensor(
                    dst, _ap(Fm, n, [[9, V], [3, 3], [0, J]]),
                    _ap(img_sw, J * n, [[3 * J, V], [0, 3], [1, J]]), op=MU)
                if n:
                    nc.vector.tensor_tensor(l_t, l_t, l_tmp, op=ADD)
            stile = sm.tile([BPG, V, 3, J], F32, tag="stile")
            nc.vector.tensor_tensor(stile, img, l_t, op=MU)
            snum = sm.tile([BPG, V, J], F32, tag="snum")
            nc.vector.tensor_tensor(snum, stile[:, :, 0], stile[:, :, 1], op=ADD)
            nc.vector.tensor_tensor(snum, snum, stile[:, :, 2], op=ADD)
            lp = sm.tile([BPG, V, 2, J], F32, tag="lp")
            lp_tmp = sm.tile([BPG, V, 2, J], F32, tag="lp_tmp")
            for n in range(3):
                dst = lp if n == 0 else lp_tmp
                nc.vector.tensor_tensor(
                    dst, _ap(Fm, 3 * n, [[9, V], [1, 2], [0, J]]),
                    _ap(img, J * n, [[3 * J, V], [0, 2], [1, J]]), op=MU)
                if n:
                    nc.vector.tensor_tensor(lp, lp, lp_tmp, op=ADD)
            q1 = sm.tile([BPG, V, 2, J], F32, tag="q1")
            nc.vector.tensor_tensor(q1, l_t[:, :, 0:2], l_t[:, :, 0:2], op=MU)
            q2 = sm.tile([BPG, V, 2, J], F32, tag="q2")
            nc.vector.tensor_tensor(q2, lp, lp, op=MU)
            nc.vector.tensor_tensor(q1, q1, q2, op=ADD)
            div = sm.tile([BPG, V, J], F32, tag="div")
            nc.vector.tensor_tensor(div, q1[:, :, 0], q1[:, :, 1], op=ADD)
            nsq = sm.tile([BPG, V, J], F32, tag="nsq")
            nc.vector.tensor_tensor(nsq, snum, snum, op=MU)
            nc.vector.tensor_scalar(div, div, EPS, None, op0=ADD)
            rdiv = sm.tile([BPG, V, J], F32, tag="rdiv")
            nc.vector.reciprocal(rdiv, div)
            nc.vector.tensor_tensor(nsq, nsq, rdiv, op=MU)
            dist = sm.tile([BPG, V, J], F32, tag="dist")
            nc.scalar.activation(out=dist, in_=nsq, func=ACT.Sqrt,
                                 bias=zb[0:BPG], scale=1.0)

            score = sm.tile([BPG, V, J], F32, tag="score")
            nc.vector.tensor_tensor(score, conf, dist, op=SUB)
            score_sw = sm.tile([BPG, V, J], F32, tag="score_sw")
            nc.vector.tensor_copy(score_sw[:, 0], score[:, 1])
            nc.vector.tensor_copy(score_sw[:, 1], score[:, 0])
            sd = sm.tile([BPG, V, J], F32, tag="sd")
            nc.vector.tensor_tensor(sd, score, score_sw, op=SUB)
            esd = sm.tile([BPG, V, J], F32, tag="esd")
            nc.scalar.activation(out=esd, in_=sd, func=ACT.Exp,
                                 bias=zb[0:BPG], scale=-1.0)
            nc.vector.tensor_scalar(esd, esd, 1.0, None, op0=ADD)
            vw = sm.tile([BPG, V, J], F32, tag="vw")
            nc.vector.reciprocal(vw, esd)
            c_t = sm.tile([BPG, V, J], F32, tag="c_t")
            nc.vector.tensor_tensor(c_t, vw, inv_mv, op=MU)

            # broadcast c: gather to one row, PE outer product, evac as bf16
            c_row = sm.tile([1, SPG * J], F32, tag="c_row")
            nc.sync.dma_start(out=c_row, in_=c_t)
            p_bc = ps.tile([128, SPG * J], F32, tag="aux")
            nc.tensor.matmul(p_bc, ones_row, c_row, start=True, stop=True)
            c_bc = sm.tile([128, SPG * J], F32, tag="c_bc")
            nc.scalar.copy(out=c_bc, in_=p_bc)

            # ---- fusion + stage 2 ----
            e2_g = []
            JA = 13  # DVE columns; gpsimd gets the rest
            for kb in range(BPG):
                b = BPG * g + kb
                k0, k1 = 2 * kb, 2 * kb + 1
                fused = fpool.tile([128, J, W], BF16)
                tmpf = fpool.tile([128, J, W], BF16)
                cb0a = _ap(c_bc, k0 * J, [[1, JA], [0, W]])
                cb1a = _ap(c_bc, k1 * J, [[1, JA], [0, W]])
                cb0b = _ap(c_bc, k0 * J + JA, [[1, J - JA], [0, W]])
                cb1b = _ap(c_bc, k1 * J + JA, [[1, J - JA], [0, W]])
                nc.vector.tensor_tensor(fused[:, 0:JA], cb0a,
                                        hm_g[:, k0, 0:JA], op=MU)
                nc.vector.tensor_tensor(tmpf[:, 0:JA], cb1a,
                                        hm_g[:, k1, 0:JA], op=MU)
                nc.vector.tensor_tensor(fused[:, 0:JA], fused[:, 0:JA],
                                        tmpf[:, 0:JA], op=ADD)
                nc.gpsimd.tensor_tensor(fused[:, JA:J], hm_g[:, k0, JA:J],
                                        cb0b, op=MU)
                nc.gpsimd.tensor_tensor(tmpf[:, JA:J], hm_g[:, k1, JA:J],
                                        cb1b, op=MU)
                nc.gpsimd.tensor_tensor(fused[:, JA:J], fused[:, JA:J],
                                        tmpf[:, JA:J], op=ADD)
                ff = fused.rearrange("p j w -> p (j w)")
                nc.scalar.dma_start(
                    out=out_hm[2 * b].rearrange("h j w -> h (j w)"), in_=ff)
                nc.scalar.dma_start(
                    out=out_hm[2 * b + 1].rearrange("h j w -> h (j w)"), in_=ff)
                e2 = e2pool.tile([128, J, W], BF16, tag=f"e2{kb}")
                nc.scalar.activation(out=e2, in_=fused, func=ACT.Exp,
                                     bias=zb, scale=TINV)
                e2_g.append(e2.rearrange("p j w -> p (j w)"))

            m2g = big.tile([2 * BPG, J, W], F32, tag="m2g")
            m2gf = m2g.rearrange("p j w -> p (j w)")
            for c0 in range(0, FD, 512):
                c1 = min(c0 + 512, FD)
                p_m2 = ps.tile([2 * BPG, 512], F32, tag="m1")
                for kb in range(BPG):
                    nc.tensor.matmul(p_m2[:, 0:c1 - c0], lhs4_bf(kb),
                                     e2_g[kb][:, c0:c1],
                                     start=(kb == 0), stop=(kb == BPG - 1))
                nc.scalar.copy(out=m2gf[:, c0:c1], in_=p_m2[:, 0:c1 - c0])
            p_t2 = ps.tile([128, J, 2 * BPG], F32, tag="ptj")
            for j in range(J):
                nc.tensor.transpose(p_t2[:, j], m2g[:, j, :],
                                    iden[0:2 * BPG, 0:2 * BPG])
            csT2 = big.tile([128, J, 2 * BPG], F32, tag="csT2")
            nc.scalar.copy(out=csT2, in_=p_t2)
            p_p2b = ps.tile([2, J, BPG, 2], F32, tag="p2")
            nc.tensor.matmul(p_p2b.rearrange("p j b r -> p (j b r)"), lhs1,
                             csT2.rearrange("p j c -> p (j c)"),
                             start=True, stop=True)
            sums2b = sm.tile([2, BPG, 2, J], F32, tag="sums2b")
            nc.scalar.copy(out=_ap(sums2b, 0, [[1, J], [2 * J, BPG], [J, 2]]),
                           in_=p_p2b)

            S2 = sm.tile([BPG, J], F32, tag="S2")
            y2n = sm.tile([BPG, J], F32, tag="y2n")
            x2n = sm.tile([BPG, J], F32, tag="x2n")
            nc.sync.dma_start(out=S2, in_=_ap(sums2b[0:1], 0,
                                              [[2 * J, BPG], [1, J]]))
            nc.sync.dma_start(out=y2n, in_=_ap(sums2b[0:1], J,
                                               [[2 * J, BPG], [1, J]]))
            nc.sync.dma_start(out=x2n, in_=_ap(sums2b[1:2], 0,
                                               [[2 * J, BPG], [1, J]]))
            rS2 = sm.tile([BPG, J], F32, tag="rS2")
            nc.vector.reciprocal(rS2, S2)
            imgc = sm.tile([BPG, 2, J], F32, tag="imgc")
            nc.vector.scalar_tensor_tensor(imgc[:, 0], x2n, 4.0, rS2,
                                           op0=MU, op1=MU)
            nc.vector.scalar_tensor_tensor(imgc[:, 1], y2n, 4.0, rS2,
                                           op0=MU, op1=MU)
            nc.sync.dma_start(out=out_img[BPG * g:BPG * (g + 1), 0], in_=imgc)
            nc.sync.dma_start(out=out_img[BPG * g:BPG * (g + 1), 1], in_=imgc)

    nc.finalize()
    return nc


def _make_cst(apk, apt, latk, latt):
    cst = np.zeros((128, C_END), dtype=np.float32)
    cst[:, C_LHS1] = 1.0
    cst[:, C_LHS1 + 1] = np.arange(128, dtype=np.float32)
    cst[:, C_IDEN:C_IDEN + 128] = np.eye(128, dtype=np.float32)
    for k in range(4):
        cst[:, C_L8 + 8 * k + 2 * k] = 1.0
        cst[:, C_L8 + 8 * k + 2 * k + 1] = np.arange(128, dtype=np.float32)
    for k in range(2):
        cst[:, C_L4 + 4 * k + 2 * k] = 1.0
        cst[:, C_L4 + 4 * k + 2 * k + 1] = np.arange(128, dtype=np.float32)
    for g in range(NG):
        bs = slice(BPG * g, BPG * (g + 1))
        cst[0:BPG, C_K + 18 * g:C_K + 18 * g + 9] = apk[bs].reshape(BPG, 9)
        cst[0:BPG, C_K + 18 * g + 9:C_K + 18 * (g + 1)] = latk[bs].reshape(BPG, 9)
        cst[0:BPG, C_T + 32 * g:C_T + 32 * g + 16] = apt[bs].reshape(BPG, 16)
        cst[0:BPG, C_T + 32 * g + 16:C_T + 32 * (g + 1)] = latt[bs].reshape(BPG, 16)
    return cst


_NC_CACHE = []
LAST_RESULTS = None


def kernel(origin_hms, AP_K, AP_T, LAT_K, LAT_T):
    global LAST_RESULTS
    from concourse.bass_utils import run_bass_kernel_spmd
    if not _NC_CACHE:
        _NC_CACHE.append(build_nc())
    nc = _NC_CACHE[0]
    f32c = lambda a: np.ascontiguousarray(np.asarray(a), dtype=np.float32)
    hms_t = np.ascontiguousarray(
        np.transpose(np.asarray(origin_hms, dtype=np.float32),
                     (0, 2, 1, 3))).astype(NPBF)  # [B*V, H, J, W] bf16
    in_maps = []
    for c in range(NC_):
        bs = slice(BPC * c, BPC * (c + 1))
        in_maps.append({
            "hms": np.ascontiguousarray(hms_t[S * c:S * (c + 1)]),
            "cst": _make_cst(f32c(AP_K[bs]), f32c(AP_T[bs]),
                             f32c(LAT_K[bs]), f32c(LAT_T[bs])),
        })
    trace = os.environ.get("BASS_KERNEL_TRACE", "0") == "1"
    res = run_bass_kernel_spmd(nc, in_maps, core_ids=list(range(NC_)), trace=trace)
    LAST_RESULTS = res
    img2 = np.empty((B, V, 2, J), dtype=np.float32)
    fused = np.empty((B * V, J, H, W), dtype=np.float32)
    for c in range(NC_):
        img2[BPC * c:BPC * (c + 1)] = res.results[c]["out_img"]
        oh = np.asarray(res.results[c]["out_hm"]).astype(np.float32)
        fused[S * c:S * (c + 1)] = np.transpose(oh, (0, 2, 1, 3))
    return img2, fused


# revision 27
# speedup vs baseline: 1.1612x; 1.1612x over previous
"""AdafuseNet multi-view heatmap fusion kernel for 8 TRN2 NeuronCores.

Pure data parallel: 32 batches sharded 4-per-core (8 bv-slices of
(17,128,128) heatmaps per core). v2: bf16 datapath (rel err ~2e-3,
gate is 2e-2), host-side [S,H,J,W] transpose so all big DMAs are
contiguous, two-batch-group pipeline to hide the per-group stats /
camera-math serial chain, bf16 full-rate PE matmuls.

Per core, per group g (batches {2g, 2g+1} = slices 4g..4g+3):
  stage 1: exp(hm/T) -> per-joint column sums via PE matmuls,
           per-joint max via DVE reduce + PE transpose
  tiny math: 3x3 camera inverses, fundamental matrices, epipolar
           distances, view-weight sigmoid - strided DVE ops on [2,2,*]
  fusion:  fused = c0*hm0 + c1*hm1 (per-joint scalars), bf16 out
  stage 2: soft-argmax on fused -> output coords
"""
import os
import sys

for _p in (
    "/root/.axon_site",
    "/root/.axon_site/_ro/trn_rl_repo",
    "/root/.axon_site/_ro/pypackages",
    "/opt/trn_rl_repo",
    "/opt/pypackages",
):
    if os.path.isdir(_p) and _p not in sys.path:
        sys.path.append(_p)

import numpy as np
import ml_dtypes
import concourse.bass as bass
import concourse.tile as tile
from concourse import bacc
from concourse import mybir
from concourse.alu_op_type import AluOpType
from contextlib import ExitStack

B, V, J, H, W = 32, 2, 17, 128, 128
NC_ = 8
BPC = B // NC_        # 4 batches per core
S = BPC * V           # 8 bv-slices per core
NG = 2                # batch groups per core
BPG = BPC // NG       # 2 batches per group
SPG = BPG * V         # 4 slices per group
TINV = 20.0           # 1 / softmax_temp
EPS = 1e-12
F32 = mybir.dt.float32
BF16 = mybir.dt.bfloat16
NPBF = ml_dtypes.bfloat16
FD = J * W            # 2176 free elems per slice
X = mybir.AxisListType.X

# cst column layout (fp32 tensor; cols 0:CBF also cast to a bf16 twin)
C_LHS1 = 0            # 2 cols: [ones | arange]
C_IDEN = 2            # 128 cols: eye(128)
C_L8 = 130            # 4 slots x 8 cols: M1 masked lhsT (pair at 2k)
C_L4 = 162            # 2 slots x 4 cols: stage2 masked lhsT
CBF = 170             # end of bf16-twin region
C_K = 170             # rows 0-1: K per group: g0 18, g1 18
C_T = 206             # rows 0-1: T per group: g0 32, g1 32
C_END = 270


def _ap(base, off, dims):
    """Custom free-dim AP on a tile: keep partition entry, replace free dims.
    dims = [[step, count], ...] in elements relative to base's offset."""
    b = base[:] if not isinstance(base, bass.AP) else base
    return bass.AP(tensor=b.tensor, offset=b.offset + off, ap=[list(b.ap[0])] + dims)


def build_nc():
    nc = bacc.Bacc()
    hms = nc.declare_dram_parameter("hms", [S, H, J, W], BF16, isOutput=False)
    cst = nc.declare_dram_parameter("cst", [128, C_END], F32, isOutput=False)
    out_hm = nc.declare_dram_parameter("out_hm", [S, H, J, W], BF16, isOutput=True)
    out_img = nc.declare_dram_parameter("out_img", [BPC, V, 2, J], F32, isOutput=True)

    MU, ADD, SUB, MX, GT = (AluOpType.mult, AluOpType.add, AluOpType.subtract,
                            AluOpType.max, AluOpType.is_gt)
    ACT = mybir.ActivationFunctionType

    with tile.TileContext(nc) as tc, ExitStack() as ctx:
        consts = ctx.enter_context(tc.tile_pool(name="consts", bufs=1))
        big = ctx.enter_context(tc.tile_pool(name="big", bufs=2))
        epool = ctx.enter_context(tc.tile_pool(name="epool", bufs=1))
        fpool = ctx.enter_context(tc.tile_pool(name="fpool", bufs=3))
        e2pool = ctx.enter_context(tc.tile_pool(name="e2pool", bufs=2))
        sm = ctx.enter_context(tc.tile_pool(name="sm", bufs=2))
        ps = ctx.enter_context(tc.tile_pool(name="ps", bufs=2, space="PSUM"))

        cst_sb = consts.tile([128, C_END], F32)
        nc.sync.dma_start(out=cst_sb, in_=cst[:])
        cst_bf = consts.tile([128, CBF], BF16)
        nc.scalar.copy(out=cst_bf, in_=cst_sb[:, 0:CBF])
        lhs1 = cst_sb[:, C_LHS1:C_LHS1 + 2]
        iden = cst_sb[:, C_IDEN:C_IDEN + 128]
        iden_bf = cst_bf[:, C_IDEN:C_IDEN + 128]
        lhs8_bf = lambda k: cst_bf[:, C_L8 + 8 * k:C_L8 + 8 * (k + 1)]
        lhs4_bf = lambda k: cst_bf[:, C_L4 + 4 * k:C_L4 + 4 * (k + 1)]
        zb = consts.tile([128, 1], F32)
        nc.vector.memset(zb, 0.0)
        ones_row = consts.tile([1, 128], F32)
        nc.vector.memset(ones_row, 1.0)

        hmpool = ctx.enter_context(tc.tile_pool(name="hmpool", bufs=1))
        hm_gs, mxc_gs = [], []
        for g in range(NG):
            hm_g = hmpool.tile([128, SPG, J, W], BF16, tag=f"hm{g}")
            maxcol_g = hmpool.tile([128, SPG, 18], BF16, tag=f"mxc{g}")
            nc.vector.memset(maxcol_g[:, :, J:18], 0.0)
            for k in range(SPG):
                s = SPG * g + k
                nc.sync.dma_start(out=hm_g[:, k].rearrange("p j w -> p (j w)"),
                                  in_=hms[s].rearrange("h j w -> h (j w)"))
            hm_gs.append(hm_g)
            mxc_gs.append(maxcol_g)
        e_gs, m1g_gs = [], []
        for g in range(NG):
            # ---- phase A: exp, maxcol, M1 passes ----
            hm_g, maxcol_g = hm_gs[g], mxc_gs[g]
            e_g = []
            for k in range(SPG):
                e_s = epool.tile([128, J, W], BF16, tag=f"e{g}_{k}")
                nc.scalar.activation(out=e_s, in_=hm_g[:, k], func=ACT.Exp,
                                     bias=zb, scale=TINV)
                nc.vector.tensor_reduce(out=maxcol_g[:, k, 0:J], in_=hm_g[:, k],
                                        axis=X, op=MX)
                e_g.append(e_s.rearrange("p j w -> p (j w)"))
            m1g = big.tile([2 * SPG, J, W], F32, tag="m1g")
            m1gf = m1g.rearrange("p j w -> p (j w)")
            for c0 in range(0, FD, 512):
                c1 = min(c0 + 512, FD)
                p_m1 = ps.tile([2 * SPG, 512], F32, tag="m1")
                for k in range(SPG):
                    nc.tensor.matmul(p_m1[:, 0:c1 - c0], lhs8_bf(k),
                                     e_g[k][:, c0:c1],
                                     start=(k == 0), stop=(k == SPG - 1))
                nc.scalar.copy(out=m1gf[:, c0:c1], in_=p_m1[:, 0:c1 - c0])

            e_gs.append(e_g)
            m1g_gs.append(m1g)

        cbc_gs = []
        for g in range(NG):
            # ---- phase B: stats chain + camera math ----
            hm_g, maxcol_g = hm_gs[g], mxc_gs[g]
            m1g = m1g_gs[g]
            # maxv over partitions: PE-transpose [128, 72] -> reduce -> [1, 72]
            mc_g = maxcol_g.rearrange("p s j -> p (s j)")
            mt = ps.tile([SPG * 18, 128], BF16, tag="aux")
            nc.tensor.transpose(mt, mc_g, iden_bf)
            mred = sm.tile([SPG * 18, 1], F32, tag="mred")
            nc.vector.tensor_reduce(out=mred, in_=mt, axis=X, op=MX)
            tree1 = sm.tile([1, SPG * 18], F32, tag="tree1")
            nc.sync.dma_start(out=tree1, in_=mred)

            # per-joint transposes + M3
            p_t = ps.tile([128, J, 2 * SPG], F32, tag="ptj")
            for j in range(J):
                nc.tensor.transpose(p_t[:, j], m1g[:, j, :],
                                    iden[0:2 * SPG, 0:2 * SPG])
            csT = big.tile([128, J, 2 * SPG], F32, tag="csT")
            nc.scalar.copy(out=csT, in_=p_t)
            p_p2 = ps.tile([2, J, SPG, 2], F32, tag="p2")
            nc.tensor.matmul(p_p2.rearrange("p j s r -> p (j s r)"), lhs1,
                             csT.rearrange("p j s -> p (j s)"),
                             start=True, stop=True)
            sums2 = sm.tile([2, SPG, 2, J], F32, tag="sums2")
            nc.scalar.copy(out=_ap(sums2, 0, [[1, J], [2 * J, SPG], [J, 2]]),
                           in_=p_p2)

            # gathers to [2, V, J] (partition = batch-in-group)
            S_t = sm.tile([BPG, V, J], F32, tag="S_t")
            ynum = sm.tile([BPG, V, J], F32, tag="ynum")
            xnum = sm.tile([BPG, V, J], F32, tag="xnum")
            conf = sm.tile([BPG, V, J], F32, tag="conf")
            nc.sync.dma_start(out=S_t, in_=_ap(sums2[0:1], 0, [[2 * J, SPG], [1, J]]))
            nc.sync.dma_start(out=ynum, in_=_ap(sums2[0:1], J, [[2 * J, SPG], [1, J]]))
            nc.sync.dma_start(out=xnum, in_=_ap(sums2[1:2], 0, [[2 * J, SPG], [1, J]]))
            nc.sync.dma_start(out=conf, in_=_ap(tree1[0:1], 0, [[18, SPG], [1, J]]))

            rS = sm.tile([BPG, V, J], F32, tag="rS")
            nc.vector.reciprocal(rS, S_t)
            img = sm.tile([BPG, V, 3, J], F32, tag="img")
            nc.vector.scalar_tensor_tensor(img[:, :, 0], xnum, 4.0, rS,
                                           op0=MU, op1=MU)
            nc.vector.scalar_tensor_tensor(img[:, :, 1], ynum, 4.0, rS,
                                           op0=MU, op1=MU)
            nc.vector.memset(img[:, :, 2], 1.0)

            # mv = where(conf > 0.01, conf, 1e6); inv_mv = 1/mv
            mask = sm.tile([BPG, V, J], F32, tag="mask")
            nc.vector.tensor_scalar(mask, conf, 0.01, None, op0=GT)
            mv = sm.tile([BPG, V, J], F32, tag="mv")
            nc.vector.tensor_tensor(mv, conf, mask, op=MU)
            mnot = sm.tile([BPG, V, J], F32, tag="mnot")
            nc.vector.tensor_scalar(mnot, mask, -1e6, 1e6, op0=MU, op1=ADD)
            nc.vector.tensor_tensor(mv, mv, mnot, op=ADD)
            inv_mv = sm.tile([BPG, V, J], F32, tag="inv_mv")
            nc.vector.reciprocal(inv_mv, mv)

            # ---- camera math on [2, 2, *] tiles ----
            K_cat = cst_sb[0:BPG, C_K + 18 * g:C_K + 18 * (g + 1)].rearrange(
                "b (v e) -> b v e", v=V)
            T_cat = cst_sb[0:BPG, C_T + 32 * g:C_T + 32 * (g + 1)].rearrange(
                "b (v e) -> b v e", v=V)

            K4 = sm.tile([BPG, V, 36], F32, tag="K4")
            src_K = _ap(K_cat, 0, [[9, V], [3, 3], [1, 3]])
            for qa, qb in ((0, 0), (0, 3), (3, 0), (3, 3)):
                nc.vector.tensor_copy(
                    _ap(K4, qa * 6 + qb, [[36, V], [6, 3], [1, 3]]), src_K)
            u1 = sm.tile([BPG, V, 9], F32, tag="u1")
            u2 = sm.tile([BPG, V, 9], F32, tag="u2")
            cof = sm.tile([BPG, V, 9], F32, tag="cof")
            st = [[36, V], [1, 3], [6, 3]]
            nc.vector.tensor_tensor(u1, _ap(K4, 7, st), _ap(K4, 14, st), op=MU)
            nc.vector.tensor_tensor(u2, _ap(K4, 8, st), _ap(K4, 13, st), op=MU)
            nc.vector.tensor_tensor(cof, u1, u2, op=SUB)
            det3 = sm.tile([BPG, V, 3], F32, tag="det3")
            nc.vector.tensor_tensor(det3, _ap(K_cat, 0, [[9, V], [1, 3]]),
                                    _ap(cof, 0, [[9, V], [3, 3]]), op=MU)
            det = sm.tile([BPG, V, 1], F32, tag="det")
            nc.vector.tensor_reduce(out=det, in_=det3, axis=X, op=ADD)
            rdet = sm.tile([BPG, V, 1], F32, tag="rdet")
            nc.vector.reciprocal(rdet, det)
            invK = sm.tile([BPG, V, 9], F32, tag="invK")
            nc.vector.scalar_tensor_tensor(invK, cof, 1.0,
                                           _ap(rdet, 0, [[1, V], [0, 9]]),
                                           op0=MU, op1=MU)
            invK_sw = sm.tile([BPG, V, 9], F32, tag="invK_sw")
            nc.vector.tensor_copy(invK_sw[:, 0], invK[:, 1])
            nc.vector.tensor_copy(invK_sw[:, 1], invK[:, 0])

            r01 = sm.tile([BPG, 9], F32, tag="r01")
            tmp9 = sm.tile([BPG, 9], F32, tag="tmp9")
            for k in range(3):
                dst = r01 if k == 0 else tmp9
                nc.vector.tensor_tensor(dst, _ap(T_cat, k, [[4, 3], [0, 3]]),
                                        _ap(T_cat, 16 + k, [[0, 3], [4, 3]]), op=MU)
                if k:
                    nc.vector.tensor_tensor(r01, r01, tmp9, op=ADD)

            t_t = sm.tile([BPG, V, 3], F32, tag="t_t")
            tmp33 = sm.tile([BPG, 3, 3], F32, tag="tmp33")
            tmp3 = sm.tile([BPG, 3, 1], F32, tag="tmp3")
            nc.vector.tensor_tensor(tmp33, _ap(r01, 0, [[3, 3], [1, 3]]),
                                    _ap(T_cat, 16 + 3, [[0, 3], [4, 3]]), op=MU)
            nc.vector.tensor_reduce(out=tmp3, in_=tmp33, axis=X, op=ADD)
            nc.vector.tensor_tensor(t_t[:, 0], _ap(T_cat, 3, [[4, 3]]),
                                    tmp3[:, :, 0], op=SUB)
            nc.vector.tensor_tensor(tmp33, _ap(r01, 0, [[1, 3], [3, 3]]),
                                    _ap(T_cat, 3, [[0, 3], [4, 3]]), op=MU)
            nc.vector.tensor_reduce(out=tmp3, in_=tmp33, axis=X, op=ADD)
            nc.vector.tensor_tensor(t_t[:, 1], _ap(T_cat, 16 + 3, [[4, 3]]),
                                    tmp3[:, :, 0], op=SUB)

            t_t2 = sm.tile([BPG, V, 6], F32, tag="t_t2")
            nc.vector.tensor_copy(_ap(t_t2, 0, [[6, V], [1, 3]]), t_t)
            nc.vector.tensor_copy(_ap(t_t2, 3, [[6, V], [1, 3]]), t_t)
            r_t2 = sm.tile([BPG, V, 18], F32, tag="r_t2")
            nc.vector.tensor_copy(_ap(r_t2, 0, [[1, 9]]), r01)
            nc.vector.tensor_copy(_ap(r_t2, 9, [[1, 9]]), r01)
            rT = _ap(r01, 0, [[1, 3], [3, 3]])
            nc.vector.tensor_copy(_ap(r_t2, 18, [[3, 3], [1, 3]]), rT)
            nc.vector.tensor_copy(_ap(r_t2, 27, [[3, 3], [1, 3]]), rT)
            Mu1 = sm.tile([BPG, V, 9], F32, tag="Mu1")
            Mu2 = sm.tile([BPG, V, 9], F32, tag="Mu2")
            Mcat = sm.tile([BPG, V, 9], F32, tag="Mcat")
            nc.vector.tensor_tensor(Mu1, _ap(t_t2, 1, [[6, V], [1, 3], [0, 3]]),
                                    _ap(r_t2, 6, [[18, V], [3, 3], [1, 3]]), op=MU)
            nc.vector.tensor_tensor(Mu2, _ap(t_t2, 2, [[6, V], [1, 3], [0, 3]]),
                                    _ap(r_t2, 3, [[18, V], [3, 3], [1, 3]]), op=MU)
            nc.vector.tensor_tensor(Mcat, Mu1, Mu2, op=SUB)

            G = sm.tile([BPG, V, 9], F32, tag="G")
            tmpG = sm.tile([BPG, V, 9], F32, tag="tmpG")
            for p_ in range(3):
                dst = G if p_ == 0 else tmpG
                nc.vector.tensor_tensor(
                    dst, _ap(invK, 3 * p_, [[9, V], [1, 3], [0, 3]]),
                    _ap(Mcat, 3 * p_, [[9, V], [0, 3], [1, 3]]), op=MU)
                if p_:
                    nc.vector.tensor_tensor(G, G, tmpG, op=ADD)
            Fm = sm.tile([BPG, V, 9], F32, tag="Fm")
            tmpF = sm.tile([BPG, V, 9], F32, tag="tmpF")
            for q in range(3):
                dst = Fm if q == 0 else tmpF
                nc.vector.tensor_tensor(
                    dst, _ap(G, q, [[9, V], [3, 3], [0, 3]]),
                    _ap(invK_sw, 3 * q, [[9, V], [0, 3], [1, 3]]), op=MU)
                if q:
                    nc.vector.tensor_tensor(Fm, Fm, tmpF, op=ADD)

            img_sw = sm.tile([BPG, V, 3, J], F32, tag="img_sw")
            nc.vector.tensor_copy(img_sw[:, 0], img[:, 1])
            nc.vector.tensor_copy(img_sw[:, 1], img[:, 0])
            l_t = sm.tile([BPG, V, 3, J], F32, tag="l_t")
            l_tmp = sm.tile([BPG, V, 3, J], F32, tag="l_tmp")
            for n in range(3):
                dst = l_t if n == 0 else l_tmp
                nc.vector.tensor_tensor(
                    dst, _ap(Fm, n, [[9, V], [3, 3], [0, J]]),
                    _ap(img_sw, J * n, [[3 * J, V], [0, 3], [1, J]]), op=MU)
                if n:
                    nc.vector.tensor_tensor(l_t, l_t, l_tmp, op=ADD)
            stile = sm.tile([BPG, V, 3, J], F32, tag="stile")
            nc.vector.tensor_tensor(stile, img, l_t, op=MU)
            snum = sm.tile([BPG, V, J], F32, tag="snum")
            nc.vector.tensor_tensor(snum, stile[:, :, 0], stile[:, :, 1], op=ADD)
            nc.vector.tensor_tensor(snum, snum, stile[:, :, 2], op=ADD)
            lp = sm.tile([BPG, V, 2, J], F32, tag="lp")
            lp_tmp = sm.tile([BPG, V, 2, J], F32, tag="lp_tmp")
            for n in range(3):
                dst = lp if n == 0 else lp_tmp
                nc.vector.tensor_tensor(
                    dst, _ap(Fm, 3 * n, [[9, V], [1, 2], [0, J]]),
                    _ap(img, J * n, [[3 * J, V], [0, 2], [1, J]]), op=MU)
                if n:
                    nc.vector.tensor_tensor(lp, lp, lp_tmp, op=ADD)
            q1 = sm.tile([BPG, V, 2, J], F32, tag="q1")
            nc.vector.tensor_tensor(q1, l_t[:, :, 0:2], l_t[:, :, 0:2], op=MU)
            q2 = sm.tile([BPG, V, 2, J], F32, tag="q2")
            nc.vector.tensor_tensor(q2, lp, lp, op=MU)
            nc.vector.tensor_tensor(q1, q1, q2, op=ADD)
            div = sm.tile([BPG, V, J], F32, tag="div")
            nc.vector.tensor_tensor(div, q1[:, :, 0], q1[:, :, 1], op=ADD)
            nsq = sm.tile([BPG, V, J], F32, tag="nsq")
            nc.vector.tensor_tensor(nsq, snum, snum, op=MU)
            nc.vector.tensor_scalar(div, div, EPS, None, op0=ADD)
            rdiv = sm.tile([BPG, V, J], F32, tag="rdiv")
            nc.vector.reciprocal(rdiv, div)
            nc.vector.tensor_tensor(nsq, nsq, rdiv, op=MU)
            dist = sm.tile([BPG, V, J], F32, tag="dist")
            nc.scalar.activation(out=dist, in_=nsq, func=ACT.Sqrt,
                                 bias=zb[0:BPG], scale=1.0)

            score = sm.tile([BPG, V, J], F32, tag="score")
            nc.vector.tensor_tensor(score, conf, dist, op=SUB)
            score_sw = sm.tile([BPG, V, J], F32, tag="score_sw")
            nc.vector.tensor_copy(score_sw[:, 0], score[:, 1])
            nc.vector.tensor_copy(score_sw[:, 1], score[:, 0])
            sd = sm.tile([BPG, V, J], F32, tag="sd")
            nc.vector.tensor_tensor(sd, score, score_sw, op=SUB)
            esd = sm.tile([BPG, V, J], F32, tag="esd")
            nc.scalar.activation(out=esd, in_=sd, func=ACT.Exp,
                                 bias=zb[0:BPG], scale=-1.0)
            nc.vector.tensor_scalar(esd, esd, 1.0, None, op0=ADD)
            vw = sm.tile([BPG, V, J], F32, tag="vw")
            nc.vector.reciprocal(vw, esd)
            c_t = sm.tile([BPG, V, J], F32, tag="c_t")
            nc.vector.tensor_tensor(c_t, vw, inv_mv, op=MU)

            # broadcast c: gather to one row, PE outer product, evac as bf16
            c_row = sm.tile([1, SPG * J], F32, tag="c_row")
            nc.sync.dma_start(out=c_row, in_=c_t)
            p_bc = ps.tile([128, SPG * J], F32, tag="aux")
            nc.tensor.matmul(p_bc, ones_row, c_row, start=True, stop=True)
            c_bc = sm.tile([128, SPG * J], F32, tag="c_bc")
            nc.scalar.copy(out=c_bc, in_=p_bc)

            cbc_gs.append(c_bc)

        e2_gs, m2g_gs = [], []
        for g in range(NG):
            # ---- phase C: fusion + stage-2 exp/matmul ----
            hm_g = hm_gs[g]
            c_bc = cbc_gs[g]
            # ---- fusion + stage 2 ----
            e2_g = []
            JA = 13  # DVE columns; gpsimd gets the rest
            for kb in range(BPG):
                b = BPG * g + kb
                k0, k1 = 2 * kb, 2 * kb + 1
                fused = fpool.tile([128, J, W], BF16)
                tmpf = fpool.tile([128, J, W], BF16)
                cb0a = _ap(c_bc, k0 * J, [[1, JA], [0, W]])
                cb1a = _ap(c_bc, k1 * J, [[1, JA], [0, W]])
                cb0b = _ap(c_bc, k0 * J + JA, [[1, J - JA], [0, W]])
                cb1b = _ap(c_bc, k1 * J + JA, [[1, J - JA], [0, W]])
                nc.vector.tensor_tensor(fused[:, 0:JA], cb0a,
                                        hm_g[:, k0, 0:JA], op=MU)
                nc.vector.tensor_tensor(tmpf[:, 0:JA], cb1a,
                                        hm_g[:, k1, 0:JA], op=MU)
                nc.vector.tensor_tensor(fused[:, 0:JA], fused[:, 0:JA],
                                        tmpf[:, 0:JA], op=ADD)
                nc.gpsimd.tensor_tensor(fused[:, JA:J], hm_g[:, k0, JA:J],
                                        cb0b, op=MU)
                nc.gpsimd.tensor_tensor(tmpf[:, JA:J], hm_g[:, k1, JA:J],
                                        cb1b, op=MU)
                nc.gpsimd.tensor_tensor(fused[:, JA:J], fused[:, JA:J],
                                        tmpf[:, JA:J], op=ADD)
                ff = fused.rearrange("p j w -> p (j w)")
                nc.scalar.dma_start(
                    out=out_hm[2 * b].rearrange("h j w -> h (j w)"), in_=ff)
                nc.scalar.dma_start(
                    out=out_hm[2 * b + 1].rearrange("h j w -> h (j w)"), in_=ff)
                e2 = e2pool.tile([128, J, W], BF16, tag=f"e2{kb}")
                nc.scalar.activation(out=e2, in_=fused, func=ACT.Exp,
                                     bias=zb, scale=TINV)
                e2_g.append(e2.rearrange("p j w -> p (j w)"))

            m2g = big.tile([2 * BPG, J, W], F32, tag="m2g")
            m2gf = m2g.rearrange("p j w -> p (j w)")
            for c0 in range(0, FD, 512):
                c1 = min(c0 + 512, FD)
                p_m2 = ps.tile([2 * BPG, 512], F32, tag="m1")
                for kb in range(BPG):
                    nc.tensor.matmul(p_m2[:, 0:c1 - c0], lhs4_bf(kb),
                                     e2_g[kb][:, c0:c1],
                                     start=(kb == 0), stop=(kb == BPG - 1))
                nc.scalar.copy(out=m2gf[:, c0:c1], in_=p_m2[:, 0:c1 - c0])
            e2_gs.append(e2_g)
            m2g_gs.append(m2g)

        for g in range(NG):
            # ---- phase D: stage-2 coords ----
            m2g = m2g_gs[g]
            p_t2 = ps.tile([128, J, 2 * BPG], F32, tag="ptj")
            for j in range(J):
                nc.tensor.transpose(p_t2[:, j], m2g[:, j, :],
                                    iden[0:2 * BPG, 0:2 * BPG])
            csT2 = big.tile([128, J, 2 * BPG], F32, tag="csT2")
            nc.scalar.copy(out=csT2, in_=p_t2)
            p_p2b = ps.tile([2, J, BPG, 2], F32, tag="p2")
            nc.tensor.matmul(p_p2b.rearrange("p j b r -> p (j b r)"), lhs1,
                             csT2.rearrange("p j c -> p (j c)"),
                             start=True, stop=True)
            sums2b = sm.tile([2, BPG, 2, J], F32, tag="sums2b")
            nc.scalar.copy(out=_ap(sums2b, 0, [[1, J], [2 * J, BPG], [J, 2]]),
                           in_=p_p2b)

            S2 = sm.tile([BPG, J], F32, tag="S2")
            y2n = sm.tile([BPG, J], F32, tag="y2n")
            x2n = sm.tile([BPG, J], F32, tag="x2n")
            nc.sync.dma_start(out=S2, in_=_ap(sums2b[0:1], 0,
                                              [[2 * J, BPG], [1, J]]))
            nc.sync.dma_start(out=y2n, in_=_ap(sums2b[0:1], J,
                                               [[2 * J, BPG], [1, J]]))
            nc.sync.dma_start(out=x2n, in_=_ap(sums2b[1:2], 0,
                                               [[2 * J, BPG], [1, J]]))
            rS2 = sm.tile([BPG, J], F32, tag="rS2")
            nc.vector.reciprocal(rS2, S2)
            imgc = sm.tile([BPG, 2, J], F32, tag="imgc")
            nc.vector.scalar_tensor_tensor(imgc[:, 0], x2n, 4.0, rS2,
                                           op0=MU, op1=MU)
            nc.vector.scalar_tensor_tensor(imgc[:, 1], y2n, 4.0, rS2,
                                           op0=MU, op1=MU)
            nc.sync.dma_start(out=out_img[BPG * g:BPG * (g + 1), 0], in_=imgc)
            nc.sync.dma_start(out=out_img[BPG * g:BPG * (g + 1), 1], in_=imgc)

    nc.finalize()
    return nc


def _make_cst(apk, apt, latk, latt):
    cst = np.zeros((128, C_END), dtype=np.float32)
    cst[:, C_LHS1] = 1.0
    cst[:, C_LHS1 + 1] = np.arange(128, dtype=np.float32)
    cst[:, C_IDEN:C_IDEN + 128] = np.eye(128, dtype=np.float32)
    for k in range(4):
        cst[:, C_L8 + 8 * k + 2 * k] = 1.0
        cst[:, C_L8 + 8 * k + 2 * k + 1] = np.arange(128, dtype=np.float32)
    for k in range(2):
        cst[:, C_L4 + 4 * k + 2 * k] = 1.0
        cst[:, C_L4 + 4 * k + 2 * k + 1] = np.arange(128, dtype=np.float32)
    for g in range(NG):
        bs = slice(BPG * g, BPG * (g + 1))
        cst[0:BPG, C_K + 18 * g:C_K + 18 * g + 9] = apk[bs].reshape(BPG, 9)
        cst[0:BPG, C_K + 18 * g + 9:C_K + 18 * (g + 1)] = latk[bs].reshape(BPG, 9)
        cst[0:BPG, C_T + 32 * g:C_T + 32 * g + 16] = apt[bs].reshape(BPG, 16)
        cst[0:BPG, C_T + 32 * g + 16:C_T + 32 * (g + 1)] = latt[bs].reshape(BPG, 16)
    return cst


_NC_CACHE = []
LAST_RESULTS = None


def kernel(origin_hms, AP_K, AP_T, LAT_K, LAT_T):
    global LAST_RESULTS
    from concourse.bass_utils import run_bass_kernel_spmd
    if not _NC_CACHE:
        _NC_CACHE.append(build_nc())
    nc = _NC_CACHE[0]
    f32c = lambda a: np.ascontiguousarray(np.asarray(a), dtype=np.float32)
    hms_t = np.ascontiguousarray(
        np.transpose(np.asarray(origin_hms, dtype=np.float32),
                     (0, 2, 1, 3))).astype(NPBF)  # [B*V, H, J, W] bf16
    in_maps = []
    for c in range(NC_):
        bs = slice(BPC * c, BPC * (c + 1))
        in_maps.append({
            "hms": np.ascontiguousarray(hms_t[S * c:S * (c + 1)]),
            "cst": _make_cst(f32c(AP_K[bs]), f32c(AP_T[bs]),
                             f32c(LAT_K[bs]), f32c(LAT_T[bs])),
        })
    trace = os.environ.get("BASS_KERNEL_TRACE", "0") == "1"
    res = run_bass_kernel_spmd(nc, in_maps, core_ids=list(range(NC_)), trace=trace)
    LAST_RESULTS = res
    img2 = np.empty((B, V, 2, J), dtype=np.float32)
    fused = np.empty((B * V, J, H, W), dtype=np.float32)
    for c in range(NC_):
        img2[BPC * c:BPC * (c + 1)] = res.results[c]["out_img"]
        oh = np.asarray(res.results[c]["out_hm"]).astype(np.float32)
        fused[S * c:S * (c + 1)] = np.transpose(oh, (0, 2, 1, 3))
    return img2, fused
